# revision 4
# baseline (speedup 1.0000x reference)
"""BFP-quantized linear kernel for Trainium2, 8-core SPMD — v3.

out = bfp_quantize(input) @ bfp_quantize(weight).T + bias
  input  [8192, 4608] f32, weight [4608, 4608] f32, bias [4608] f32
  BFP: groups of 36 contiguous elements (along rows), shared exponent
  from the group absmax, mantissas truncated toward zero to 8 bits.

v3 changes vs v2 (1287 us):
  * k-sharded weight quantization: core c quantizes w[:, 576c:576c+576]
    (host slices it contiguously), transposes to [576 k, 4608 o] and
    bounces into four per-o-quarter buffers; the AllGather for o-quarter
    j completes as soon as the first 9 o-row-tiles are quantized, so
    the matmul stream for output group j starts ~110us into the run
    instead of ~600us.
  * emission interleaved for overlap: w-quarter0 quant -> AG0 -> x
    n-half0 quant -> og0-h0 matmuls with the remaining w-tiles/AGs and
    x n-half1 quant woven between output blocks.  All engines stream
    concurrently; the PE is the pacing engine (~690us of bf16 matmul
    at the sustained 13/16-throttled clock).
  * leaner quant recipe (6 DVE passes): es = (bits>>7)&0xFF exponent
    field per element (one fused tensor_scalar), group max-reduce in
    i16, s = e7 - es, kill-mask zm16 = (s-8)>>a15 (one fused op), shift
    pair, and the kill is a bitwise AND on gpsimd (int domain - no NaN
    hazard).  Bit-identical to v2's recipe.
"""

import numpy as np

import concourse.bass as bass
import concourse.mybir as mybir
import concourse.tile as tile
from concourse import bacc
from concourse import bass_utils
from concourse.masks import make_identity

N_CORES = 8
N_ROWS, K_IN, O_OUT = 8192, 4608, 4608
NSH = N_ROWS // N_CORES   # 1024 input rows per core
KSH = K_IN // N_CORES     # 576 k columns quantized per core
GS = 36                   # BFP group size
KT = K_IN // 128          # 36 k tiles
NB = NSH // 128           # 8 n blocks per core
OB_TOT = O_OUT // 128     # 36 o blocks
OG = 4                    # o groups (AllGather chunks) of 1152 cols
OGW = O_OUT // OG         # 1152
XCH = 1152                # x quantization chunk (32 groups)

F32 = mybir.dt.float32
BF16 = mybir.dt.bfloat16
I16 = mybir.dt.int16


def _emit_quant3(nc, tpool, src, qdst, width, tag):
    """Quantize src[:, :width] (f32) into qdst[:, :width] (bf16), full 128
    rows.  Exponent-space recipe, bit-identical to the probe-validated v2:

      es   = (bits16 >> 7) & 0xFF          (element exponent field)
      e7   = group-max(es)                 (= exponent of group absmax)
      s    = e7 - es                       (mantissa bits to drop)
      zm16 = (s - 8) asr 15                (0xFFFF if s<8 else 0)
      q    = ((xh >> s) << s) & zm16       (AND on gpsimd, int domain)
    """
    ng = width // GS
    xs = src[:, :width]

    xh = tpool.tile([128, width], I16, tag=f"xh{tag}", name="xh")
    xpairs = xs.bitcast(BF16).rearrange("p (k t) -> p k t", t=2)
    nc.scalar.copy(xh[:].bitcast(BF16), xpairs[:, :, 1])

    es = tpool.tile([128, width], I16, tag=f"es{tag}", name="es")
    nc.vector.tensor_scalar(
        out=es[:], in0=xh[:], scalar1=0x7F80, scalar2=7,
        op0=mybir.AluOpType.bitwise_and,
        op1=mybir.AluOpType.logical_shift_right,
    )
    e7 = tpool.tile([128, ng], I16, tag=f"e7{tag}", name="e7")
    nc.vector.tensor_reduce(
        out=e7[:], in_=es[:].rearrange("p (g e) -> p g e", e=GS),
        axis=mybir.AxisListType.X, op=mybir.AluOpType.max,
    )
    # s = e7 - es  (in place over es)
    nc.vector.scalar_tensor_tensor(
        out=es[:].rearrange("p (g e) -> p g e", e=GS),
        in0=es[:].rearrange("p (g e) -> p g e", e=GS),
        scalar=-1,
        in1=e7[:].unsqueeze(-1).broadcast_to([128, ng, GS]),
        op0=mybir.AluOpType.mult,
        op1=mybir.AluOpType.add,
    )
    zm = tpool.tile([128, width], BF16, tag=f"zm{tag}", name="zm")
    nc.vector.tensor_scalar(
        out=zm[:], in0=es[:], scalar1=8, scalar2=None,
        op0=mybir.AluOpType.is_lt,
    )
    nc.vector.tensor_tensor(
        out=xh[:], in0=xh[:], in1=es[:],
        op=mybir.AluOpType.logical_shift_right,
    )
    nc.vector.tensor_tensor(
        out=xh[:], in0=xh[:], in1=es[:],
        op=mybir.AluOpType.logical_shift_left,
    )
    nc.gpsimd.tensor_tensor(
        out=qdst[:, :width], in0=xh[:].bitcast(BF16), in1=zm[:],
        op=mybir.AluOpType.mult,
    )


def emit_kernel(tc, nc, x_d, w_d, b_d, o_d):
    with (
        tc.tile_pool(name="dram", bufs=1, space="DRAM") as dpool,
        tc.tile_pool(name="consts", bufs=1) as cpool,
        tc.tile_pool(name="wstage", bufs=2) as wspool,
        tc.tile_pool(name="xstage", bufs=2) as xspool,
        tc.tile_pool(name="qtmps", bufs=2) as tpool,
        tc.tile_pool(name="qw", bufs=2) as wqnat,
        tc.tile_pool(name="qx", bufs=2) as xqnat,
        tc.tile_pool(name="qxt", bufs=1) as xtpool,
        tc.tile_pool(name="wstream", bufs=38) as wpool,
        tc.tile_pool(name="tstage", bufs=4) as tspool,
        tc.tile_pool(name="outs", bufs=3) as opool,
        tc.tile_pool(name="pmm", bufs=4, space="PSUM") as pmm,
        tc.tile_pool(name="ptp", bufs=3, space="PSUM") as ptp,
    ):
        ident = cpool.tile([128, 128], BF16, name="ident")
        make_identity(nc, ident[:])
        biasT = cpool.tile([128, OB_TOT], F32, name="biasT")
        nc.sync.dma_start(out=biasT[:], in_=b_d.rearrange("(o p) -> p o", p=128))

        # DRAM bounce + gathered buffers, chunked by o-quarter
        qw_boun = [
            dpool.tile([KSH, OGW], BF16, name=f"qw_boun{j}") for j in range(OG)
        ]
        qwt_g = [
            dpool.tile([K_IN, OGW], BF16, addr_space="Shared", name=f"qwt_g{j}")
            for j in range(OG)
        ]
        # resident transposed quantized input: qxT[kt] is [128 k, 1024 n]
        qxT = [xtpool.tile([128, NSH], BF16, name=f"qxT{kt}") for kt in range(KT)]

        def w_tile(i):
            """Quantize + transpose + bounce o-row-tile i (of 36) of this
            core's [4608, 576] k-shard."""
            j = i // 9          # o-quarter
            ol = (i % 9) * 128  # col offset within the quarter
            wt = wspool.tile([128, KSH], F32, tag="wst", name="wt")
            nc.sync.dma_start(out=wt[:], in_=w_d[i * 128 : (i + 1) * 128, :])
            qw = wqnat.tile([128, KSH], BF16, tag="qwn", name="qw")
            _emit_quant3(nc, tpool, wt, qw, KSH, "w")
            for t in range(5):  # k slices: 4x128 + 64
                kk = 128 if t < 4 else 64
                pt = ptp.tile([128, 128], BF16, tag="tp", name="pt")
                nc.tensor.transpose(
                    pt[:kk, :], qw[:, t * 128 : t * 128 + kk], ident[:]
                )
                st = tspool.tile([128, 128], BF16, tag="ts", name="st")
                nc.scalar.copy(st[:kk, :], pt[:kk, :])
                nc.sync.dma_start(
                    out=qw_boun[j][t * 128 : t * 128 + kk, ol : ol + 128],
                    in_=st[:kk, :],
                )

        def ag(j):
            nc.gpsimd.collective_compute(
                "AllGather",
                mybir.AluOpType.bypass,
                replica_groups=[list(range(N_CORES))],
                ins=[qw_boun[j][:].opt()],
                outs=[qwt_g[j][:].opt()],
            )

        def x_block(nb, kq):
            """Quantize x rows [128nb, 128(nb+1)) k-quarter kq and transpose
            into qxT[kt][:, nb*128:...] for kt in [18kq/2 ...)."""
            xt = xspool.tile([128, XCH], F32, tag="xst", name="xt")
            nc.sync.dma_start(
                out=xt[:],
                in_=x_d[nb * 128 : (nb + 1) * 128, kq * XCH : (kq + 1) * XCH],
            )
            qx = xqnat.tile([128, XCH], BF16, tag="qxn", name="qx")
            _emit_quant3(nc, tpool, xt, qx, XCH, "x")
            for ktl in range(XCH // 128):
                kt = kq * (XCH // 128) + ktl
                pt = ptp.tile([128, 128], BF16, tag="tp", name="pt")
                nc.tensor.transpose(pt[:], qx[:, ktl * 128 : (ktl + 1) * 128], ident[:])
                nc.scalar.copy(qxT[kt][:, nb * 128 : (nb + 1) * 128], pt[:])

        def mm_pass(j, h, interleave=None):
            """Matmuls for o-group j (9 o-blocks), n-half h.  interleave is
            an optional list of thunks; up to two are emitted after each
            o-block to weave quant/AG work into the engine queues."""
            wq = []
            for kt in range(KT):
                wqt = wpool.tile([128, OGW], BF16, tag="wq", name="wqt")
                nc.sync.dma_start(
                    out=wqt[:], in_=qwt_g[j][kt * 128 : (kt + 1) * 128, :]
                )
                wq.append(wqt)
            for obl in range(9):
                ob = j * 9 + obl
                ps = pmm.tile([128, 512], F32, tag="mm", name="ps")
                for kt in range(KT):
                    nc.tensor.matmul(
                        ps[:],
                        wq[kt][:, obl * 128 : (obl + 1) * 128],
                        qxT[kt][:, h * 512 : (h + 1) * 512],
                        start=(kt == 0),
                        stop=(kt == KT - 1),
                    )
                ot = opool.tile([128, 512], F32, tag="ot", name="ot")
                nc.scalar.activation(
                    ot[:], ps[:],
                    mybir.ActivationFunctionType.Identity,
                    bias=biasT[:, ob : ob + 1], scale=1.0,
                )
                nc.sync.dma_start(
                    out=o_d[ob * 128 : (ob + 1) * 128, h * 512 : (h + 1) * 512],
                    in_=ot[:],
                )
                if interleave:
                    for _ in range(2):
                        if interleave:
                            interleave.pop(0)()

        # ---------------- emission schedule ----------------
        # prefix: w o-quarter 0 -> AG0, then x n-half 0 (all k)
        for i in range(9):
            w_tile(i)
        ag(0)
        for nb in range(4):
            for kq in range(4):
                x_block(nb, kq)

        # og0-h0 matmuls with w-tiles 9..35 + AG1..3 woven in
        work = []
        for i in range(9, 36):
            work.append(lambda i=i: w_tile(i))
            if i in (17, 26, 35):
                work.append(lambda j=i // 9: ag(j))
        mm_pass(0, 0, interleave=work)
        while work:
            work.pop(0)()

        # og1-h0 with x n-half 1 woven in
        work = []
        for nb in range(4, 8):
            for kq in range(4):
                work.append(lambda nb=nb, kq=kq: x_block(nb, kq))
        mm_pass(1, 0, interleave=work)
        while work:
            work.pop(0)()

        mm_pass(2, 0)
        mm_pass(3, 0)
        for j in range(OG):
            mm_pass(j, 1)


_CACHED_NC = None


def _build():
    global _CACHED_NC
    if _CACHED_NC is not None:
        return _CACHED_NC
    nc = bacc.Bacc(
        "TRN2", target_bir_lowering=False, debug=False, num_devices=N_CORES
    )
    x_d = nc.dram_tensor("x", [NSH, K_IN], F32, kind="ExternalInput").ap()
    w_d = nc.dram_tensor("w", [K_IN, KSH], F32, kind="ExternalInput").ap()
    b_d = nc.dram_tensor("b", [O_OUT], F32, kind="ExternalInput").ap()
    o_d = nc.dram_tensor("o", [O_OUT, NSH], F32, kind="ExternalOutput").ap()
    with tile.TileContext(nc) as tc:
        emit_kernel(tc, nc, x_d, w_d, b_d, o_d)
    nc.compile()
    _CACHED_NC = nc
    return nc


def _ensure_axon_hooks_importable():
    import sys
    import types

    if "antenv.axon_hooks" not in sys.modules:
        try:
            import antenv.axon_hooks  # noqa: F401
        except ImportError:
            mod = types.ModuleType("antenv.axon_hooks")
            mod.get_axon_ntff_profile_hook = lambda: None
            mod.set_axon_ntff_profile_hook = lambda h: None
            sys.modules["antenv.axon_hooks"] = mod


def run_on_hw(input, weight, bias, trace=False):
    _ensure_axon_hooks_importable()
    nc = _build()
    in_maps = []
    for c in range(N_CORES):
        in_maps.append(
            {
                "x": np.ascontiguousarray(input[c * NSH : (c + 1) * NSH]),
                "w": np.ascontiguousarray(weight[:, c * KSH : (c + 1) * KSH]),
                "b": np.ascontiguousarray(bias),
            }
        )
    res = bass_utils.run_bass_kernel_spmd(
        nc, in_maps, core_ids=list(range(N_CORES)), trace=trace
    )
    out = np.empty((N_ROWS, O_OUT), dtype=np.float32)
    for c in range(N_CORES):
        out[c * NSH : (c + 1) * NSH] = res.results[c]["o"].T
    return out, res


def kernel(input, weight, bias):
    out, _ = run_on_hw(
        np.asarray(input, dtype=np.float32),
        np.asarray(weight, dtype=np.float32),
        np.asarray(bias, dtype=np.float32),
    )
    return out


# revision 10
# speedup vs baseline: 1.0522x; 1.0522x over previous
"""BFP-quantized linear kernel for Trainium2, 8-core SPMD — v3.

out = bfp_quantize(input) @ bfp_quantize(weight).T + bias
  input  [8192, 4608] f32, weight [4608, 4608] f32, bias [4608] f32
  BFP: groups of 36 contiguous elements (along rows), shared exponent
  from the group absmax, mantissas truncated toward zero to 8 bits.

v3 changes vs v2 (1287 us):
  * k-sharded weight quantization: core c quantizes w[:, 576c:576c+576]
    (host slices it contiguously), transposes to [576 k, 4608 o] and
    bounces into four per-o-quarter buffers; the AllGather for o-quarter
    j completes as soon as the first 9 o-row-tiles are quantized, so
    the matmul stream for output group j starts ~110us into the run
    instead of ~600us.
  * emission interleaved for overlap: w-quarter0 quant -> AG0 -> x
    n-half0 quant -> og0-h0 matmuls with the remaining w-tiles/AGs and
    x n-half1 quant woven between output blocks.  All engines stream
    concurrently; the PE is the pacing engine (~690us of bf16 matmul
    at the sustained 13/16-throttled clock).
  * leaner quant recipe (6 DVE passes): es = (bits>>7)&0xFF exponent
    field per element (one fused tensor_scalar), group max-reduce in
    i16, s = e7 - es, kill-mask zm16 = (s-8)>>a15 (one fused op), shift
    pair, and the kill is a bitwise AND on gpsimd (int domain - no NaN
    hazard).  Bit-identical to v2's recipe.
"""

import numpy as np

import concourse.bass as bass
import concourse.mybir as mybir
import concourse.tile as tile
from concourse import bacc
from concourse import bass_utils
from concourse.masks import make_identity

N_CORES = 8
N_ROWS, K_IN, O_OUT = 8192, 4608, 4608
NSH = N_ROWS // N_CORES   # 1024 input rows per core
KSH = K_IN // N_CORES     # 576 k columns quantized per core
GS = 36                   # BFP group size
KT = K_IN // 128          # 36 k tiles
NB = NSH // 128           # 8 n blocks per core
OB_TOT = O_OUT // 128     # 36 o blocks
OG = 4                    # o groups (AllGather chunks) of 1152 cols
OGW = O_OUT // OG         # 1152
XCH = 1152                # x quantization chunk (32 groups)

F32 = mybir.dt.float32
BF16 = mybir.dt.bfloat16
I16 = mybir.dt.int16


def _emit_quant3(nc, tpool, src, qdst, width, tag):
    """Quantize src[:, :width] (f32) into qdst[:, :width] (bf16), full 128
    rows.  Exponent-space recipe, bit-identical to the probe-validated v2:

      es   = (bits16 >> 7) & 0xFF          (element exponent field)
      e7   = group-max(es)                 (= exponent of group absmax)
      s    = e7 - es                       (mantissa bits to drop)
      zm16 = (s - 8) asr 15                (0xFFFF if s<8 else 0)
      q    = ((xh >> s) << s) & zm16       (AND on gpsimd, int domain)
    """
    ng = width // GS
    xs = src[:, :width]

    xh = tpool.tile([128, width], I16, tag=f"xh{tag}", name="xh")
    xpairs = xs.bitcast(BF16).rearrange("p (k t) -> p k t", t=2)
    nc.scalar.copy(xh[:].bitcast(BF16), xpairs[:, :, 1])

    es = tpool.tile([128, width], I16, tag=f"es{tag}", name="es")
    nc.vector.tensor_scalar(
        out=es[:], in0=xh[:], scalar1=0x7F80, scalar2=7,
        op0=mybir.AluOpType.bitwise_and,
        op1=mybir.AluOpType.logical_shift_right,
    )
    e7 = tpool.tile([128, ng], I16, tag=f"e7{tag}", name="e7")
    nc.vector.tensor_reduce(
        out=e7[:], in_=es[:].rearrange("p (g e) -> p g e", e=GS),
        axis=mybir.AxisListType.X, op=mybir.AluOpType.max,
    )
    # s = e7 - es  (in place over es)
    nc.vector.scalar_tensor_tensor(
        out=es[:].rearrange("p (g e) -> p g e", e=GS),
        in0=es[:].rearrange("p (g e) -> p g e", e=GS),
        scalar=-1,
        in1=e7[:].unsqueeze(-1).broadcast_to([128, ng, GS]),
        op0=mybir.AluOpType.mult,
        op1=mybir.AluOpType.add,
    )
    zm = tpool.tile([128, width], BF16, tag=f"zm{tag}", name="zm")
    nc.vector.tensor_scalar(
        out=zm[:], in0=es[:], scalar1=8, scalar2=None,
        op0=mybir.AluOpType.is_lt,
    )
    nc.vector.tensor_tensor(
        out=xh[:], in0=xh[:], in1=es[:],
        op=mybir.AluOpType.logical_shift_right,
    )
    nc.vector.tensor_tensor(
        out=xh[:], in0=xh[:], in1=es[:],
        op=mybir.AluOpType.logical_shift_left,
    )
    # kill-mult on DVE, NOT gpsimd: the gpsimd queue must hold only the
    # collectives — a collective blocks the queue until it completes, and
    # any quant op behind it stalls the whole pipeline via pool buffers.
    nc.vector.tensor_tensor(
        out=qdst[:, :width], in0=xh[:].bitcast(BF16), in1=zm[:],
        op=mybir.AluOpType.mult,
    )


def emit_kernel(tc, nc, x_d, w_d, b_d, o_d):
    with (
        tc.tile_pool(name="dram", bufs=1, space="DRAM") as dpool,
        tc.tile_pool(name="consts", bufs=1) as cpool,
        tc.tile_pool(name="wstage", bufs=2) as wspool,
        tc.tile_pool(name="xstage", bufs=2) as xspool,
        tc.tile_pool(name="qtmps", bufs=2) as tpool,
        tc.tile_pool(name="qw", bufs=2) as wqnat,
        tc.tile_pool(name="qx", bufs=2) as xqnat,
        tc.tile_pool(name="qxt", bufs=1) as xtpool,
        tc.tile_pool(name="wstream", bufs=38) as wpool,
        tc.tile_pool(name="tstage", bufs=4) as tspool,
        tc.tile_pool(name="outs", bufs=3) as opool,
        tc.tile_pool(name="pmm", bufs=4, space="PSUM") as pmm,
        tc.tile_pool(name="ptp", bufs=3, space="PSUM") as ptp,
    ):
        ident = cpool.tile([128, 128], BF16, name="ident")
        make_identity(nc, ident[:])
        biasT = cpool.tile([128, OB_TOT], F32, name="biasT")
        nc.sync.dma_start(out=biasT[:], in_=b_d.rearrange("(o p) -> p o", p=128))

        # DRAM bounce + gathered buffers, chunked by o-quarter
        qw_boun = [
            dpool.tile([KSH, OGW], BF16, name=f"qw_boun{j}") for j in range(OG)
        ]
        qwt_g = [
            dpool.tile([K_IN, OGW], BF16, addr_space="Shared", name=f"qwt_g{j}")
            for j in range(OG)
        ]
        # resident transposed quantized input: qxT[kt] is [128 k, 1024 n]
        qxT = [xtpool.tile([128, NSH], BF16, name=f"qxT{kt}") for kt in range(KT)]

        def w_tile(i):
            """Quantize + transpose + bounce o-row-tile i (of 36) of this
            core's [4608, 576] k-shard."""
            j = i // 9          # o-quarter
            ol = (i % 9) * 128  # col offset within the quarter
            wt = wspool.tile([128, KSH], F32, tag="wst", name="wt")
            nc.sync.dma_start(out=wt[:], in_=w_d[i * 128 : (i + 1) * 128, :])
            qw = wqnat.tile([128, KSH], BF16, tag="qwn", name="qw")
            _emit_quant3(nc, tpool, wt, qw, KSH, "w")
            for t in range(5):  # k slices: 4x128 + 64
                kk = 128 if t < 4 else 64
                pt = ptp.tile([128, 128], BF16, tag="tp", name="pt")
                nc.tensor.transpose(
                    pt[:kk, :], qw[:, t * 128 : t * 128 + kk], ident[:]
                )
                st = tspool.tile([128, 128], BF16, tag="ts", name="st")
                nc.scalar.copy(st[:kk, :], pt[:kk, :])
                nc.sync.dma_start(
                    out=qw_boun[j][t * 128 : t * 128 + kk, ol : ol + 128],
                    in_=st[:kk, :],
                )

        def ag(j):
            nc.gpsimd.collective_compute(
                "AllGather",
                mybir.AluOpType.bypass,
                replica_groups=[list(range(N_CORES))],
                ins=[qw_boun[j][:].opt()],
                outs=[qwt_g[j][:].opt()],
            )

        def x_block(nb, kq):
            """Quantize x rows [128nb, 128(nb+1)) k-quarter kq and transpose
            into qxT[kt][:, nb*128:...] for kt in [18kq/2 ...)."""
            xt = xspool.tile([128, XCH], F32, tag="xst", name="xt")
            nc.sync.dma_start(
                out=xt[:],
                in_=x_d[nb * 128 : (nb + 1) * 128, kq * XCH : (kq + 1) * XCH],
            )
            qx = xqnat.tile([128, XCH], BF16, tag="qxn", name="qx")
            _emit_quant3(nc, tpool, xt, qx, XCH, "x")
            for ktl in range(XCH // 128):
                kt = kq * (XCH // 128) + ktl
                pt = ptp.tile([128, 128], BF16, tag="tp", name="pt")
                nc.tensor.transpose(pt[:], qx[:, ktl * 128 : (ktl + 1) * 128], ident[:])
                nc.scalar.copy(qxT[kt][:, nb * 128 : (nb + 1) * 128], pt[:])

        def mm_pass(j, passes):
            """Matmuls for o-group j (9 o-blocks).  passes is a list of
            (c0, cw, interleave) column passes sharing one load of the
            gathered weights.  interleave is an optional list of thunks;
            up to two are emitted after each o-block to weave quant/AG
            work into the engine queues."""
            wq = []
            for kt in range(KT):
                wqt = wpool.tile([128, OGW], BF16, tag="wq", name="wqt")
                nc.sync.dma_start(
                    out=wqt[:], in_=qwt_g[j][kt * 128 : (kt + 1) * 128, :]
                )
                wq.append(wqt)
            for c0, cw, interleave in passes:
                for obl in range(9):
                    ob = j * 9 + obl
                    ps = pmm.tile([128, 512], F32, tag="mm", name="ps")
                    for kt in range(KT):
                        nc.tensor.matmul(
                            ps[:, :cw],
                            wq[kt][:, obl * 128 : (obl + 1) * 128],
                            qxT[kt][:, c0 : c0 + cw],
                            start=(kt == 0),
                            stop=(kt == KT - 1),
                        )
                    ot = opool.tile([128, 512], F32, tag="ot", name="ot")
                    nc.scalar.activation(
                        ot[:, :cw], ps[:, :cw],
                        mybir.ActivationFunctionType.Identity,
                        bias=biasT[:, ob : ob + 1], scale=1.0,
                    )
                    nc.sync.dma_start(
                        out=o_d[ob * 128 : (ob + 1) * 128, c0 : c0 + cw],
                        in_=ot[:, :cw],
                    )
                    if interleave:
                        for _ in range(2):
                            if interleave:
                                interleave.pop(0)()

        # ---------------- emission schedule ----------------
        # Target timeline (DVE-paced prefix, PE-paced from ~85us):
        #   DVE: w0-8 | x nb0-1 | w9-17 | x nb2 | x nb3 | w18-35 | x nb4-7
        #   AGs fire as their o-quarter's bounce completes; gpsimd queue
        #   holds nothing else, so a pending AG never stalls quant.
        #   PE: og0 over n-cols 0:256 starts ~85us (needs only AG0+nb01),
        #   then 256:384 (nb2), 384:512 (nb3), og1-3 h0 at 512 wide, then
        #   the four h1 passes.
        def wt(i):
            return lambda: w_tile(i)

        def xb(nb, kq):
            return lambda: x_block(nb, kq)

        def run(work):
            while work:
                work.pop(0)()

        for i in range(9):
            w_tile(i)
        ag(0)
        for nb in range(2):
            for kq in range(4):
                x_block(nb, kq)

        work_a = [wt(i) for i in range(9, 18)] + [lambda: ag(1)]
        work_a += [xb(2, kq) for kq in range(4)]
        work_b = [xb(3, kq) for kq in range(4)] + [wt(i) for i in range(18, 21)]
        work_c = [wt(i) for i in range(21, 27)] + [lambda: ag(2)]
        mm_pass(0, [(0, 256, work_a), (256, 128, work_b), (384, 128, work_c)])
        run(work_a), run(work_b), run(work_c)

        work = [wt(i) for i in range(27, 36)] + [lambda: ag(3)]
        mm_pass(1, [(0, 512, work)])
        run(work)

        work = [xb(nb, kq) for nb in range(4, 6) for kq in range(4)]
        mm_pass(2, [(0, 512, work)])
        run(work)

        work = [xb(nb, kq) for nb in range(6, 8) for kq in range(4)]
        mm_pass(3, [(0, 512, work)])
        run(work)

        for j in range(OG):
            mm_pass(j, [(512, 512, None)])


_CACHED_NC = None


def _build():
    global _CACHED_NC
    if _CACHED_NC is not None:
        return _CACHED_NC
    nc = bacc.Bacc(
        "TRN2", target_bir_lowering=False, debug=False, num_devices=N_CORES
    )
    x_d = nc.dram_tensor("x", [NSH, K_IN], F32, kind="ExternalInput").ap()
    w_d = nc.dram_tensor("w", [K_IN, KSH], F32, kind="ExternalInput").ap()
    b_d = nc.dram_tensor("b", [O_OUT], F32, kind="ExternalInput").ap()
    o_d = nc.dram_tensor("o", [O_OUT, NSH], F32, kind="ExternalOutput").ap()
    with tile.TileContext(nc) as tc:
        emit_kernel(tc, nc, x_d, w_d, b_d, o_d)
    nc.compile()
    _CACHED_NC = nc
    return nc


def _ensure_axon_hooks_importable():
    import sys
    import types

    if "antenv.axon_hooks" not in sys.modules:
        try:
            import antenv.axon_hooks  # noqa: F401
        except ImportError:
            mod = types.ModuleType("antenv.axon_hooks")
            mod.get_axon_ntff_profile_hook = lambda: None
            mod.set_axon_ntff_profile_hook = lambda h: None
            sys.modules["antenv.axon_hooks"] = mod


def run_on_hw(input, weight, bias, trace=False):
    _ensure_axon_hooks_importable()
    nc = _build()
    in_maps = []
    for c in range(N_CORES):
        in_maps.append(
            {
                "x": np.ascontiguousarray(input[c * NSH : (c + 1) * NSH]),
                "w": np.ascontiguousarray(weight[:, c * KSH : (c + 1) * KSH]),
                "b": np.ascontiguousarray(bias),
            }
        )
    res = bass_utils.run_bass_kernel_spmd(
        nc, in_maps, core_ids=list(range(N_CORES)), trace=trace
    )
    out = np.empty((N_ROWS, O_OUT), dtype=np.float32)
    for c in range(N_CORES):
        out[c * NSH : (c + 1) * NSH] = res.results[c]["o"].T
    return out, res


def kernel(input, weight, bias):
    out, _ = run_on_hw(
        np.asarray(input, dtype=np.float32),
        np.asarray(weight, dtype=np.float32),
        np.asarray(bias, dtype=np.float32),
    )
    return out
